# revision 22
# baseline (speedup 1.0000x reference)
"""MeanAggregatorSparse on 8 Trainium2 NeuronCores.

out = concat(self_feat, segment_mean(nbr_feat, idx)) @ W

Strategy: shard NODES across the 8 cores (6272 nodes/core = 49 windows of
128). Edges are bucketed host-side to the core/window owning their target
node (this is the sharding step - each core receives exactly the edges it
needs, so no collective is required). On device, each 128-edge tile builds a
weighted one-hot matrix oh[e, n] = (idx_local[e] == n) * (1/count[idx[e]])
with a single DVE tensor_scalar op, and the PE contracts

  S_T[feat, nodes] += feat_tile[edges, feat].T @ oh[edges, nodes]

accumulating a full 128-node window in PSUM. The weighted one-hot folds the
mean division into the matmul, and the transposed accumulator is exactly the
lhsT layout needed by the output GEMM, so no transposes appear anywhere:

  out[nodes, :] = aggT.T @ W_bot + selfT.T @ W_top     (accumulated in PSUM)

Each window slot j has its own edge capacity T_j = max edge count over the 8
cores for that slot (rounded up to 128) so the SPMD program is uniform while
padding stays small. Feats are laid out partition-major per window so each
SBUF partition receives one contiguous chunk per window DMA. The kernel is
HBM-bandwidth-bound: ~49 MB/core at ~358 GB/s.
"""

import numpy as np

P = 128
N_NODES = 50000
D_FEAT = 128
OUT_DIM = 128
N_CORES = 8
WPC = 49                        # node windows per core
NPC = WPC * P                   # nodes per core (6272)
NODES_PAD = N_CORES * NPC       # 50176
N_WIN = N_CORES * WPC           # 392

_prog_cache = {}


def _build_program(key, repeat=1):
    """Build the SPMD Bass program. key = (NTs, rems): NTs[j] = number of
    128-edge tiles for window slot j, rems[j] = lanes used by the last
    (partial) tile; the feats block for slot j holds exactly
    128*(NTs[j]-1) + rems[j] rows (no tile-rounding padding). Same for
    every core. repeat > 1 unrolls the body N times (same result) - used by
    bench.py to measure device time as a slope."""
    import concourse.mybir as mybir
    import concourse.tile as tile
    from concourse import bacc
    from contextlib import ExitStack

    f32 = mybir.dt.float32
    NTs, rems = [list(x) for x in key]
    C = sum(NTs)                       # total tiles per core
    rows = [0] * (WPC + 1)             # feats row offset per window
    cols = [0] * (WPC + 1)             # meta tile-column offset per window
    for j, nt in enumerate(NTs):
        rows[j + 1] = rows[j] + P * (nt - 1) + rems[j]
        cols[j + 1] = cols[j] + nt

    nc = bacc.Bacc(
        "TRN2", target_bir_lowering=False, debug=False, num_devices=N_CORES
    )
    feats = nc.declare_dram_parameter("feats", [rows[WPC], D_FEAT], f32, isOutput=False)
    meta = nc.declare_dram_parameter("meta", [P, C * 2], f32, isOutput=False)
    selfT = nc.declare_dram_parameter("selfT", [P, NPC], f32, isOutput=False)
    wmat = nc.declare_dram_parameter("wmat", [2 * D_FEAT, OUT_DIM], f32, isOutput=False)
    iota = nc.declare_dram_parameter("iota", [P, P], f32, isOutput=False)
    outp = nc.declare_dram_parameter("outp", [NPC, OUT_DIM], f32, isOutput=True)

    with tile.TileContext(nc) as tc, ExitStack() as ctx:
        # const loads + output stores ride the ACT HWDGE ring so the SP ring
        # streams nothing but the big feats window loads.
        const = ctx.enter_context(tc.tile_pool(name="const", bufs=1))
        selft = const.tile([P, NPC], f32)
        nc.scalar.dma_start(selft[:], selfT[:])
        wtop = const.tile([P, OUT_DIM], f32, tag="wtop")
        nc.scalar.dma_start(wtop[:], wmat[0:P, :])
        wbot = const.tile([P, OUT_DIM], f32, tag="wbot")
        nc.scalar.dma_start(wbot[:], wmat[P : 2 * P, :])
        metat = const.tile([P, C * 2], f32)
        nc.scalar.dma_start(metat[:], meta[:])
        iotat = const.tile([P, P], f32)
        nc.scalar.dma_start(iotat[:], iota[:])

        featp = ctx.enter_context(tc.tile_pool(name="featp", bufs=4))
        ohp = ctx.enter_context(tc.tile_pool(name="ohp", bufs=6))
        aggp = ctx.enter_context(tc.tile_pool(name="aggp", bufs=2))
        obp = ctx.enter_context(tc.tile_pool(name="obp", bufs=2))
        psS_p = ctx.enter_context(tc.tile_pool(name="psS", bufs=2, space="PSUM"))
        psO_p = ctx.enter_context(tc.tile_pool(name="psO", bufs=2, space="PSUM"))

        eq = mybir.AluOpType.is_equal
        mul = mybir.AluOpType.mult
        NT_MAX = max(NTs)

        # Unwritten lanes of partial tiles are masked by a zero one-hot
        # column, but 0 * NaN = NaN, so scrub the rotating feat buffers once
        # at startup in case SBUF powers up with NaN bit patterns.
        for _ in range(4):
            t = featp.tile([P, NT_MAX * D_FEAT], f32, tag="ft")
            nc.gpsimd.memset(t[:], 0)

        for j in [jj for _ in range(repeat) for jj in range(WPC)]:
            NT = NTs[j]
            rem = rems[j]
            # ft is allocated at the max size so every window shares one
            # buffer tag; only the first NT*D_FEAT columns are loaded/used.
            ft = featp.tile([P, NT_MAX * D_FEAT], f32, tag="ft")
            # partitions < rem carry NT rows each; partitions >= rem carry
            # NT-1 rows each (the partial tile only fills lanes < rem)
            split = rows[j] + rem * NT
            src = feats[rows[j] : split, :].rearrange("(p k) f -> p (k f)", p=rem)
            nc.sync.dma_start(ft[:rem, : NT * D_FEAT], src)
            if rem < P and NT > 1:
                src2 = feats[split : rows[j + 1], :].rearrange(
                    "(p k) f -> p (k f)", p=P - rem
                )
                nc.sync.dma_start(ft[rem:, : (NT - 1) * D_FEAT], src2)
            psS = psS_p.tile([P, P], f32)
            for k in range(NT):
                oh = ohp.tile([P, P], f32)
                c = (cols[j] + k) * 2
                nc.vector.tensor_scalar(
                    out=oh[:],
                    in0=iotat[:],
                    scalar1=metat[:, c : c + 1],
                    scalar2=metat[:, c + 1 : c + 2],
                    op0=eq,
                    op1=mul,
                )
                nc.tensor.matmul(
                    psS[:],
                    lhsT=ft[:, k * D_FEAT : (k + 1) * D_FEAT],
                    rhs=oh[:],
                    start=(k == 0),
                    stop=(k == NT - 1),
                )
            aggT = aggp.tile([P, P], f32)
            nc.scalar.copy(aggT[:], psS[:])
            psO = psO_p.tile([P, OUT_DIM], f32)
            nc.tensor.matmul(psO[:], lhsT=aggT[:], rhs=wbot[:], start=True, stop=False)
            nc.tensor.matmul(
                psO[:],
                lhsT=selft[:, j * P : (j + 1) * P],
                rhs=wtop[:],
                start=False,
                stop=True,
            )
            ob = obp.tile([P, OUT_DIM], f32)
            nc.scalar.copy(ob[:], psO[:])
            nc.scalar.dma_start(outp[j * P : (j + 1) * P, :], ob[:])

    nc.compile()
    return nc


def _prep_inputs(self_feat, nbr_feat, relation_src_indices, W):
    """Host-side sharding: bucket edges by target window, pad each window
    slot to the max count over the 8 cores (rounded to 128), and build the
    per-core input arrays."""
    idx = np.asarray(relation_src_indices).astype(np.int64)
    feat = np.ascontiguousarray(np.asarray(nbr_feat, dtype=np.float32))
    E = idx.shape[0]

    win = idx >> 7                     # global window id, 0..390
    counts_win = np.bincount(win, minlength=N_WIN)
    # per-slot capacity: max edge count over the 8 cores for that slot
    slot_max = np.maximum(1, counts_win.reshape(N_CORES, WPC).max(axis=0))
    NTs = -(-slot_max // P)            # tiles per slot
    rems = slot_max - P * (NTs - 1)    # lanes used by the last tile
    C = int(NTs.sum())
    rows = np.zeros(WPC + 1, np.int64)
    rows[1:] = np.cumsum(slot_max)
    cols = np.zeros(WPC + 1, np.int64)
    cols[1:] = np.cumsum(NTs)
    rows_per_core = int(rows[WPC])

    order = np.argsort(win, kind="stable")
    sw = win[order]
    si = idx[order]
    starts = np.zeros(N_WIN, np.int64)
    starts[1:] = np.cumsum(counts_win)[:-1]
    rank = np.arange(E, dtype=np.int64) - starts[sw]

    core = sw // WPC
    slot = sw % WPC
    nt_e = NTs[slot]
    rem_e = rems[slot]
    p_e = rank % P
    k_e = rank // P
    # feats: partition-major within each window block; partitions < rem get
    # nt rows, partitions >= rem get nt-1 rows
    row_in_block = np.where(
        p_e < rem_e,
        p_e * nt_e + k_e,
        rem_e * nt_e + (p_e - rem_e) * (nt_e - 1) + k_e,
    )
    dest_feat = core * rows_per_core + rows[slot] + row_in_block
    # meta: tile-major (= rank order) position within the core's tile list
    dest_meta = core * (C * P) + cols[slot] * P + rank

    feats_packed = np.zeros((N_CORES * rows_per_core, D_FEAT), np.float32)
    feats_packed[dest_feat] = feat[order]

    lidx = np.full(N_CORES * C * P, -1.0, np.float32)
    lidx[dest_meta] = (si - (sw << 7)).astype(np.float32)

    cnt_node = np.bincount(idx, minlength=NODES_PAD).astype(np.float32)
    wv = np.zeros(N_CORES * C * P, np.float32)
    wv[dest_meta] = 1.0 / cnt_node[si]

    # meta[core, p, (cols[j]+k)*2 + {0,1}] = lidx / weight of tile column
    lidx_t = lidx.reshape(N_CORES, C, P).transpose(0, 2, 1)
    wv_t = wv.reshape(N_CORES, C, P).transpose(0, 2, 1)
    meta = np.empty((N_CORES, P, C * 2), np.float32)
    meta[:, :, 0::2] = lidx_t
    meta[:, :, 1::2] = wv_t

    selfp = np.zeros((NODES_PAD, D_FEAT), np.float32)
    selfp[:N_NODES] = np.asarray(self_feat, dtype=np.float32)
    selfT = np.ascontiguousarray(
        selfp.reshape(N_CORES, NPC, D_FEAT).transpose(0, 2, 1)
    )

    wrep = np.ascontiguousarray(np.asarray(W, dtype=np.float32))
    iota = np.ascontiguousarray(np.tile(np.arange(P, dtype=np.float32), (P, 1)))

    feats_c = feats_packed.reshape(N_CORES, rows_per_core, D_FEAT)
    in_maps = [
        {
            "feats": np.ascontiguousarray(feats_c[c]),
            "meta": np.ascontiguousarray(meta[c]),
            "selfT": selfT[c],
            "wmat": wrep,
            "iota": iota,
        }
        for c in range(N_CORES)
    ]
    key = (tuple(int(x) for x in NTs), tuple(int(x) for x in rems))
    return key, in_maps


def kernel(self_feat, nbr_feat, relation_src_indices, W):
    from concourse.bass_utils import run_bass_kernel_spmd

    key, in_maps = _prep_inputs(self_feat, nbr_feat, relation_src_indices, W)

    nc = _prog_cache.get(key)
    if nc is None:
        nc = _build_program(key)
        _prog_cache[key] = nc

    res = run_bass_kernel_spmd(nc, in_maps, list(range(N_CORES)))
    out = np.concatenate([res.results[c]["outp"] for c in range(N_CORES)], axis=0)
    return np.ascontiguousarray(out[:N_NODES])


# revision 23
# speedup vs baseline: 4.6661x; 4.6661x over previous
"""MeanAggregatorSparse on 8 Trainium2 NeuronCores.

out = concat(self_feat, segment_mean(nbr_feat, idx)) @ W

Strategy: shard NODES across the 8 cores (6272 nodes/core = 49 windows of
128). Edges are bucketed host-side to the core/window owning their target
node (this is the sharding step - each core receives exactly the edges it
needs, so no collective is required). On device, each 128-edge tile builds a
weighted one-hot matrix oh[e, n] = (idx_local[e] == n) * (1/count[idx[e]])
with a single DVE tensor_scalar op, and the PE contracts

  S_T[feat, nodes] += feat_tile[edges, feat].T @ oh[edges, nodes]

accumulating a full 128-node window in PSUM. The weighted one-hot folds the
mean division into the matmul, and the transposed accumulator is exactly the
lhsT layout needed by the output GEMM, so no transposes appear anywhere:

  out[nodes, :] = aggT.T @ W_bot + selfT.T @ W_top     (accumulated in PSUM)

Each window slot j has its own edge capacity T_j = max edge count over the 8
cores for that slot (rounded up to 128) so the SPMD program is uniform while
padding stays small. Feats are laid out partition-major per window so each
SBUF partition receives one contiguous chunk per window DMA. The kernel is
HBM-bandwidth-bound: ~49 MB/core at ~358 GB/s.
"""

import numpy as np

P = 128
N_NODES = 50000
D_FEAT = 128
OUT_DIM = 128
N_CORES = 8
WPC = 49                        # node windows per core
NPC = WPC * P                   # nodes per core (6272)
NODES_PAD = N_CORES * NPC       # 50176
N_WIN = N_CORES * WPC           # 392

_prog_cache = {}


def _build_program(key, repeat=1):
    """Build the SPMD Bass program. key = (NTs, rems): NTs[j] = number of
    128-edge tiles for window slot j, rems[j] = lanes used by the last
    (partial) tile; the feats block for slot j holds exactly
    128*(NTs[j]-1) + rems[j] rows (no tile-rounding padding). Same for
    every core. repeat > 1 unrolls the body N times (same result) - used by
    bench.py to measure device time as a slope."""
    import concourse.mybir as mybir
    import concourse.tile as tile
    from concourse import bacc
    from contextlib import ExitStack

    f32 = mybir.dt.float32
    NTs, rems = [list(x) for x in key]
    C = sum(NTs)                       # total tiles per core
    rows = [0] * (WPC + 1)             # feats row offset per window
    cols = [0] * (WPC + 1)             # meta tile-column offset per window
    for j, nt in enumerate(NTs):
        rows[j + 1] = rows[j] + P * (nt - 1) + rems[j]
        cols[j + 1] = cols[j] + nt

    nc = bacc.Bacc(
        "TRN2", target_bir_lowering=False, debug=False, num_devices=N_CORES
    )
    feats = nc.declare_dram_parameter("feats", [rows[WPC], D_FEAT], f32, isOutput=False)
    meta = nc.declare_dram_parameter("meta", [P, C * 2], f32, isOutput=False)
    selfT = nc.declare_dram_parameter("selfT", [P, NPC], f32, isOutput=False)
    wmat = nc.declare_dram_parameter("wmat", [2 * D_FEAT, OUT_DIM], f32, isOutput=False)
    iota = nc.declare_dram_parameter("iota", [P, P], f32, isOutput=False)
    outp = nc.declare_dram_parameter("outp", [NPC, OUT_DIM], f32, isOutput=True)

    with tile.TileContext(nc) as tc, ExitStack() as ctx:
        # const loads + output stores ride the ACT HWDGE ring so the SP ring
        # streams nothing but the big feats window loads.
        const = ctx.enter_context(tc.tile_pool(name="const", bufs=1))
        selft = const.tile([P, NPC], f32)
        nc.scalar.dma_start(selft[:], selfT[:])
        wtop = const.tile([P, OUT_DIM], f32, tag="wtop")
        nc.scalar.dma_start(wtop[:], wmat[0:P, :])
        wbot = const.tile([P, OUT_DIM], f32, tag="wbot")
        nc.scalar.dma_start(wbot[:], wmat[P : 2 * P, :])
        metat = const.tile([P, C * 2], f32)
        nc.scalar.dma_start(metat[:], meta[:])
        iotat = const.tile([P, P], f32)
        nc.scalar.dma_start(iotat[:], iota[:])

        featp = ctx.enter_context(tc.tile_pool(name="featp", bufs=4))
        ohp = ctx.enter_context(tc.tile_pool(name="ohp", bufs=6))
        aggp = ctx.enter_context(tc.tile_pool(name="aggp", bufs=2))
        obp = ctx.enter_context(tc.tile_pool(name="obp", bufs=2))
        psS_p = ctx.enter_context(tc.tile_pool(name="psS", bufs=2, space="PSUM"))
        psO_p = ctx.enter_context(tc.tile_pool(name="psO", bufs=2, space="PSUM"))

        eq = mybir.AluOpType.is_equal
        mul = mybir.AluOpType.mult
        NT_MAX = max(NTs)

        # Unwritten lanes of partial tiles are masked by a zero one-hot
        # column, but 0 * NaN = NaN, so scrub the rotating feat buffers once
        # at startup in case SBUF powers up with NaN bit patterns.
        for _ in range(4):
            t = featp.tile([P, NT_MAX * D_FEAT], f32, tag="ft")
            nc.gpsimd.memset(t[:], 0)

        for j in [jj for _ in range(repeat) for jj in range(WPC)]:
            NT = NTs[j]
            rem = rems[j]
            # ft is allocated at the max size so every window shares one
            # buffer tag; only the first NT*D_FEAT columns are loaded/used.
            ft = featp.tile([P, NT_MAX * D_FEAT], f32, tag="ft")
            # full tiles: one 128-partition rectangle (full DMA port width);
            # partial tile: a small [rem, 128] tail block
            split = rows[j] + P * (NT - 1)
            if NT > 1:
                src = feats[rows[j] : split, :].rearrange(
                    "(p k) f -> p (k f)", p=P
                )
                nc.sync.dma_start(ft[:, : (NT - 1) * D_FEAT], src)
            nc.sync.dma_start(
                ft[:rem, (NT - 1) * D_FEAT : NT * D_FEAT],
                feats[split : rows[j + 1], :],
            )
            psS = psS_p.tile([P, P], f32)
            for k in range(NT):
                oh = ohp.tile([P, P], f32)
                c = (cols[j] + k) * 2
                nc.vector.tensor_scalar(
                    out=oh[:],
                    in0=iotat[:],
                    scalar1=metat[:, c : c + 1],
                    scalar2=metat[:, c + 1 : c + 2],
                    op0=eq,
                    op1=mul,
                )
                nc.tensor.matmul(
                    psS[:],
                    lhsT=ft[:, k * D_FEAT : (k + 1) * D_FEAT],
                    rhs=oh[:],
                    start=(k == 0),
                    stop=(k == NT - 1),
                )
            aggT = aggp.tile([P, P], f32)
            nc.scalar.copy(aggT[:], psS[:])
            psO = psO_p.tile([P, OUT_DIM], f32)
            nc.tensor.matmul(psO[:], lhsT=aggT[:], rhs=wbot[:], start=True, stop=False)
            nc.tensor.matmul(
                psO[:],
                lhsT=selft[:, j * P : (j + 1) * P],
                rhs=wtop[:],
                start=False,
                stop=True,
            )
            ob = obp.tile([P, OUT_DIM], f32)
            nc.scalar.copy(ob[:], psO[:])
            nc.scalar.dma_start(outp[j * P : (j + 1) * P, :], ob[:])

    nc.compile()
    return nc


def _prep_inputs(self_feat, nbr_feat, relation_src_indices, W):
    """Host-side sharding: bucket edges by target window, pad each window
    slot to the max count over the 8 cores (rounded to 128), and build the
    per-core input arrays."""
    idx = np.asarray(relation_src_indices).astype(np.int64)
    feat = np.ascontiguousarray(np.asarray(nbr_feat, dtype=np.float32))
    E = idx.shape[0]

    win = idx >> 7                     # global window id, 0..390
    counts_win = np.bincount(win, minlength=N_WIN)
    # per-slot capacity: max edge count over the 8 cores for that slot
    slot_max = np.maximum(1, counts_win.reshape(N_CORES, WPC).max(axis=0))
    NTs = -(-slot_max // P)            # tiles per slot
    rems = slot_max - P * (NTs - 1)    # lanes used by the last tile
    C = int(NTs.sum())
    rows = np.zeros(WPC + 1, np.int64)
    rows[1:] = np.cumsum(slot_max)
    cols = np.zeros(WPC + 1, np.int64)
    cols[1:] = np.cumsum(NTs)
    rows_per_core = int(rows[WPC])

    order = np.argsort(win, kind="stable")
    sw = win[order]
    si = idx[order]
    starts = np.zeros(N_WIN, np.int64)
    starts[1:] = np.cumsum(counts_win)[:-1]
    rank = np.arange(E, dtype=np.int64) - starts[sw]

    core = sw // WPC
    slot = sw % WPC
    nt_e = NTs[slot]
    rem_e = rems[slot]
    p_e = rank % P
    k_e = rank // P
    # feats: full tiles form a partition-major rectangle (row p*(nt-1)+k),
    # the partial tile is a row-major tail block at the end of the window
    row_in_block = np.where(
        k_e < nt_e - 1,
        p_e * (nt_e - 1) + k_e,
        P * (nt_e - 1) + p_e,
    )
    dest_feat = core * rows_per_core + rows[slot] + row_in_block
    # meta: tile-major (= rank order) position within the core's tile list
    dest_meta = core * (C * P) + cols[slot] * P + rank

    feats_packed = np.zeros((N_CORES * rows_per_core, D_FEAT), np.float32)
    feats_packed[dest_feat] = feat[order]

    lidx = np.full(N_CORES * C * P, -1.0, np.float32)
    lidx[dest_meta] = (si - (sw << 7)).astype(np.float32)

    cnt_node = np.bincount(idx, minlength=NODES_PAD).astype(np.float32)
    wv = np.zeros(N_CORES * C * P, np.float32)
    wv[dest_meta] = 1.0 / cnt_node[si]

    # meta[core, p, (cols[j]+k)*2 + {0,1}] = lidx / weight of tile column
    lidx_t = lidx.reshape(N_CORES, C, P).transpose(0, 2, 1)
    wv_t = wv.reshape(N_CORES, C, P).transpose(0, 2, 1)
    meta = np.empty((N_CORES, P, C * 2), np.float32)
    meta[:, :, 0::2] = lidx_t
    meta[:, :, 1::2] = wv_t

    selfp = np.zeros((NODES_PAD, D_FEAT), np.float32)
    selfp[:N_NODES] = np.asarray(self_feat, dtype=np.float32)
    selfT = np.ascontiguousarray(
        selfp.reshape(N_CORES, NPC, D_FEAT).transpose(0, 2, 1)
    )

    wrep = np.ascontiguousarray(np.asarray(W, dtype=np.float32))
    iota = np.ascontiguousarray(np.tile(np.arange(P, dtype=np.float32), (P, 1)))

    feats_c = feats_packed.reshape(N_CORES, rows_per_core, D_FEAT)
    in_maps = [
        {
            "feats": np.ascontiguousarray(feats_c[c]),
            "meta": np.ascontiguousarray(meta[c]),
            "selfT": selfT[c],
            "wmat": wrep,
            "iota": iota,
        }
        for c in range(N_CORES)
    ]
    key = (tuple(int(x) for x in NTs), tuple(int(x) for x in rems))
    return key, in_maps


def kernel(self_feat, nbr_feat, relation_src_indices, W):
    from concourse.bass_utils import run_bass_kernel_spmd

    key, in_maps = _prep_inputs(self_feat, nbr_feat, relation_src_indices, W)

    nc = _prog_cache.get(key)
    if nc is None:
        nc = _build_program(key)
        _prog_cache[key] = nc

    res = run_bass_kernel_spmd(nc, in_maps, list(range(N_CORES)))
    out = np.concatenate([res.results[c]["outp"] for c in range(N_CORES)], axis=0)
    return np.ascontiguousarray(out[:N_NODES])


# revision 25
# speedup vs baseline: 4.8839x; 1.0467x over previous
"""MeanAggregatorSparse on 8 Trainium2 NeuronCores.

out = concat(self_feat, segment_mean(nbr_feat, idx)) @ W

Strategy: shard NODES across the 8 cores (6272 nodes/core = 49 windows of
128). Edges are bucketed host-side to the core/window owning their target
node (this is the sharding step - each core receives exactly the edges it
needs, so no collective is required). On device, each 128-edge tile builds a
weighted one-hot matrix oh[e, n] = (idx_local[e] == n) * (1/count[idx[e]])
with a single DVE tensor_scalar op, and the PE contracts

  S_T[feat, nodes] += feat_tile[edges, feat].T @ oh[edges, nodes]

accumulating a full 128-node window in PSUM. The weighted one-hot folds the
mean division into the matmul, and the transposed accumulator is exactly the
lhsT layout needed by the output GEMM, so no transposes appear anywhere:

  out[nodes, :] = aggT.T @ W_bot + selfT.T @ W_top     (accumulated in PSUM)

Each window slot j has its own edge capacity T_j = max edge count over the 8
cores for that slot (rounded up to 128) so the SPMD program is uniform while
padding stays small. Feats are laid out partition-major per window so each
SBUF partition receives one contiguous chunk per window DMA. The kernel is
HBM-bandwidth-bound: ~49 MB/core at ~358 GB/s.
"""

import numpy as np

P = 128
N_NODES = 50000
D_FEAT = 128
OUT_DIM = 128
N_CORES = 8
WPC = 49                        # node windows per core
NPC = WPC * P                   # nodes per core (6272)
NODES_PAD = N_CORES * NPC       # 50176
N_WIN = N_CORES * WPC           # 392

_prog_cache = {}


def _build_program(key, repeat=1):
    """Build the SPMD Bass program. key = (NTs, rems): NTs[j] = number of
    128-edge tiles for window slot j, rems[j] = lanes used by the last
    (partial) tile; the feats block for slot j holds exactly
    128*(NTs[j]-1) + rems[j] rows (no tile-rounding padding). Same for
    every core. repeat > 1 unrolls the body N times (same result) - used by
    bench.py to measure device time as a slope."""
    import concourse.mybir as mybir
    import concourse.tile as tile
    from concourse import bacc
    from contextlib import ExitStack

    f32 = mybir.dt.float32
    NTs, rems = [list(x) for x in key]
    C = sum(NTs)                       # total tiles per core
    rows = [0] * (WPC + 1)             # feats row offset per window
    cols = [0] * (WPC + 1)             # meta tile-column offset per window
    for j, nt in enumerate(NTs):
        rows[j + 1] = rows[j] + P * (nt - 1) + rems[j]
        cols[j + 1] = cols[j] + nt

    nc = bacc.Bacc(
        "TRN2", target_bir_lowering=False, debug=False, num_devices=N_CORES
    )
    feats = nc.declare_dram_parameter("feats", [rows[WPC], D_FEAT], f32, isOutput=False)
    meta = nc.declare_dram_parameter("meta", [P, C * 2], f32, isOutput=False)
    selfT = nc.declare_dram_parameter("selfT", [P, NPC], f32, isOutput=False)
    wmat = nc.declare_dram_parameter("wmat", [2 * D_FEAT, OUT_DIM], f32, isOutput=False)
    iota = nc.declare_dram_parameter("iota", [P, P], f32, isOutput=False)
    outp = nc.declare_dram_parameter("outp", [NPC, OUT_DIM], f32, isOutput=True)

    with tile.TileContext(nc) as tc, ExitStack() as ctx:
        # const loads + output stores ride the ACT HWDGE ring so the SP ring
        # streams nothing but the big feats window loads.
        const = ctx.enter_context(tc.tile_pool(name="const", bufs=1))
        selft = const.tile([P, NPC], f32)
        nc.scalar.dma_start(selft[:], selfT[:])
        wtop = const.tile([P, OUT_DIM], f32, tag="wtop")
        nc.scalar.dma_start(wtop[:], wmat[0:P, :])
        wbot = const.tile([P, OUT_DIM], f32, tag="wbot")
        nc.scalar.dma_start(wbot[:], wmat[P : 2 * P, :])
        metat = const.tile([P, C * 2], f32)
        nc.scalar.dma_start(metat[:], meta[:])
        iotat = const.tile([P, P], f32)
        nc.scalar.dma_start(iotat[:], iota[:])

        featp = ctx.enter_context(tc.tile_pool(name="featp", bufs=4))
        ohp = ctx.enter_context(tc.tile_pool(name="ohp", bufs=6))
        aggp = ctx.enter_context(tc.tile_pool(name="aggp", bufs=2))
        obp = ctx.enter_context(tc.tile_pool(name="obp", bufs=2))
        psS_p = ctx.enter_context(tc.tile_pool(name="psS", bufs=2, space="PSUM"))
        psO_p = ctx.enter_context(tc.tile_pool(name="psO", bufs=2, space="PSUM"))

        eq = mybir.AluOpType.is_equal
        mul = mybir.AluOpType.mult
        NT_MAX = max(NTs)

        # Unwritten lanes of partial tiles are masked by a zero one-hot
        # column, but 0 * NaN = NaN, so scrub the rotating feat buffers once
        # at startup in case SBUF powers up with NaN bit patterns.
        for _ in range(4):
            t = featp.tile([P, NT_MAX * D_FEAT], f32, tag="ft")
            nc.gpsimd.memset(t[:], 0)

        for j in [jj for _ in range(repeat) for jj in range(WPC)]:
            NT = NTs[j]
            rem = rems[j]
            # ft is allocated at the max size so every window shares one
            # buffer tag; only the first NT*D_FEAT columns are loaded/used.
            ft = featp.tile([P, NT_MAX * D_FEAT], f32, tag="ft")
            # full tiles: one 128-partition rectangle (full DMA port width);
            # partial tile: a small [rem, 128] tail block
            split = rows[j] + P * (NT - 1)
            if NT > 1:
                src = feats[rows[j] : split, :].rearrange(
                    "(p k) f -> p (k f)", p=P
                )
                nc.sync.dma_start(ft[:, : (NT - 1) * D_FEAT], src)
            nc.sync.dma_start(
                ft[:rem, (NT - 1) * D_FEAT : NT * D_FEAT],
                feats[split : rows[j + 1], :],
            )
            psS = psS_p.tile([P, P], f32)
            for k in range(NT):
                oh = ohp.tile([P, P], f32)
                c = (cols[j] + k) * 2
                nc.vector.tensor_scalar(
                    out=oh[:],
                    in0=iotat[:],
                    scalar1=metat[:, c : c + 1],
                    scalar2=metat[:, c + 1 : c + 2],
                    op0=eq,
                    op1=mul,
                )
                nc.tensor.matmul(
                    psS[:],
                    lhsT=ft[:, k * D_FEAT : (k + 1) * D_FEAT],
                    rhs=oh[:],
                    start=(k == 0),
                    stop=(k == NT - 1),
                )
            aggT = aggp.tile([P, P], f32)
            nc.scalar.copy(aggT[:], psS[:])
            psO = psO_p.tile([P, OUT_DIM], f32)
            nc.tensor.matmul(psO[:], lhsT=aggT[:], rhs=wbot[:], start=True, stop=False)
            nc.tensor.matmul(
                psO[:],
                lhsT=selft[:, j * P : (j + 1) * P],
                rhs=wtop[:],
                start=False,
                stop=True,
            )
            ob = obp.tile([P, OUT_DIM], f32)
            nc.scalar.copy(ob[:], psO[:])
            nc.scalar.dma_start(outp[j * P : (j + 1) * P, :], ob[:])

    nc.compile()
    return nc


def _prep_inputs(self_feat, nbr_feat, relation_src_indices, W):
    """Host-side sharding: bucket edges by target window, pad each window
    slot to the max count over the 8 cores (rounded to 128), and build the
    per-core input arrays."""
    idx = np.asarray(relation_src_indices).astype(np.int64)
    feat = np.ascontiguousarray(np.asarray(nbr_feat, dtype=np.float32))
    E = idx.shape[0]

    win = idx >> 7                     # global window id, 0..390
    counts_win = np.bincount(win, minlength=N_WIN)
    # per-slot capacity: max edge count over the 8 cores for that slot
    slot_max = np.maximum(1, counts_win.reshape(N_CORES, WPC).max(axis=0))
    NTs = -(-slot_max // P)            # tiles per slot
    rems = slot_max - P * (NTs - 1)    # lanes used by the last tile
    C = int(NTs.sum())
    rows = np.zeros(WPC + 1, np.int64)
    rows[1:] = np.cumsum(slot_max)
    cols = np.zeros(WPC + 1, np.int64)
    cols[1:] = np.cumsum(NTs)
    rows_per_core = int(rows[WPC])

    order = np.argsort(win, kind="stable")
    sw = win[order]
    si = idx[order]
    starts = np.zeros(N_WIN, np.int64)
    starts[1:] = np.cumsum(counts_win)[:-1]
    rank = np.arange(E, dtype=np.int64) - starts[sw]

    core = sw // WPC
    slot = sw % WPC
    nt_e = NTs[slot]
    rem_e = rems[slot]
    p_e = rank % P
    k_e = rank // P
    # feats: full tiles form a partition-major rectangle (row p*(nt-1)+k),
    # the partial tile is a row-major tail block at the end of the window
    row_in_block = np.where(
        k_e < nt_e - 1,
        p_e * (nt_e - 1) + k_e,
        P * (nt_e - 1) + p_e,
    )
    dest_feat = core * rows_per_core + rows[slot] + row_in_block
    # meta: tile-major (= rank order) position within the core's tile list
    dest_meta = core * (C * P) + cols[slot] * P + rank

    feats_packed = np.zeros((N_CORES * rows_per_core, D_FEAT), np.float32)
    feats_packed[dest_feat] = feat[order]

    lidx = np.full(N_CORES * C * P, -1.0, np.float32)
    lidx[dest_meta] = (si - (sw << 7)).astype(np.float32)

    cnt_node = np.bincount(idx, minlength=NODES_PAD).astype(np.float32)
    wv = np.zeros(N_CORES * C * P, np.float32)
    wv[dest_meta] = 1.0 / cnt_node[si]

    # meta[core, p, (cols[j]+k)*2 + {0,1}] = lidx / weight of tile column
    lidx_t = lidx.reshape(N_CORES, C, P).transpose(0, 2, 1)
    wv_t = wv.reshape(N_CORES, C, P).transpose(0, 2, 1)
    meta = np.empty((N_CORES, P, C * 2), np.float32)
    meta[:, :, 0::2] = lidx_t
    meta[:, :, 1::2] = wv_t

    selfp = np.zeros((NODES_PAD, D_FEAT), np.float32)
    selfp[:N_NODES] = np.asarray(self_feat, dtype=np.float32)
    selfT = np.ascontiguousarray(
        selfp.reshape(N_CORES, NPC, D_FEAT).transpose(0, 2, 1)
    )

    wrep = np.ascontiguousarray(np.asarray(W, dtype=np.float32))
    iota = np.ascontiguousarray(np.tile(np.arange(P, dtype=np.float32), (P, 1)))

    feats_c = feats_packed.reshape(N_CORES, rows_per_core, D_FEAT)
    in_maps = [
        {
            "feats": np.ascontiguousarray(feats_c[c]),
            "meta": np.ascontiguousarray(meta[c]),
            "selfT": selfT[c],
            "wmat": wrep,
            "iota": iota,
        }
        for c in range(N_CORES)
    ]
    key = (tuple(int(x) for x in NTs), tuple(int(x) for x in rems))
    return key, in_maps


def kernel(self_feat, nbr_feat, relation_src_indices, W):
    from concourse.bass_utils import run_bass_kernel_spmd

    key, in_maps = _prep_inputs(self_feat, nbr_feat, relation_src_indices, W)

    nc = _prog_cache.get(key)
    if nc is None:
        nc = _build_program(key)
        _prog_cache[key] = nc

    res = run_bass_kernel_spmd(nc, in_maps, list(range(N_CORES)))
    out = np.concatenate([res.results[c]["outp"] for c in range(N_CORES)], axis=0)
    return np.ascontiguousarray(out[:N_NODES])
